# revision 13
# baseline (speedup 1.0000x reference)
"""MoE layer (nn_MoELayer_6923487282556) on 8 Trainium2 cores.

Strategy: 2D-parallel FFN, perfectly load-balanced under SPMD.
  Host router: fp64 numpy router (logits, softmax, top-2, renormalized
    combine weights). Decides the dispatch; error ~1e-7, far below the
    top-2/top-3 logit gap, so routing matches the fp32 reference.
  Dispatch: tokens of every expert are split into 4 equal quarters
    (padded by <=3 zero-weight tokens); the hidden dim (3072) is split
    into 2 halves. Core (q, h) processes quarter q of EVERY expert's
    tokens against hidden half h of that expert's weights. Per-core row
    count is identical by construction -> no expert-imbalance padding
    (the old expert-per-core layout ran max(cnt_e)=4255 rows per core;
    this runs sum_e ceil(cnt_e/4) / ... = ~4100 row-equivalents).
  Device: silu(x@gwT) * (x@uwT) @ dwT over the half-hidden slice,
    scaled by the combine weight, fp16 with fp32 PSUM accumulation.
    Output is the half-hidden partial sum of the expert output.
  Host: adds the two hidden-half partials and scatter-adds the per-pair
    contributions into the output.

Everything is transposed ([feature, token] layout) so no on-device
transposes are needed; all DRAM operands are laid out so each SBUF
partition reads contiguous bursts.
"""

import numpy as np

import concourse.bass as bass
import concourse.tile as tile
from concourse import bacc, mybir
from concourse.bass_utils import run_bass_kernel_spmd

F16 = mybir.dt.float16
F32 = mybir.dt.float32
AF = mybir.ActivationFunctionType

N_CORES = 8
B, L, D = 4, 4096, 2048
N = B * L            # 16384 tokens
E = 8                # experts
H = 3072             # ffn hidden
NQ = 4               # token quarters
NH = 2               # hidden halves
HH = H // NH         # 1536 hidden rows per half
KC = D // 128        # 16 contraction chunks over D
MH = HH // 128       # 12 hidden m-blocks per half
DC = D // 128        # 16 output chunks over D

# When set (by test.py) each launch's execution is wrapped with the axon
# NTFF profile hook and traces land in PROFILE_DIR/launch{1,2}.
PROFILE_DIR = None

_cache = {}


def _run(nc, in_maps, tag):
    core_ids = list(range(N_CORES))
    if PROFILE_DIR is None:
        try:
            return run_bass_kernel_spmd(nc, in_maps, core_ids).results
        except Exception:
            # transient NRT_EXEC_UNIT_UNRECOVERABLE wedges have been
            # observed; a single retry recovers them
            return run_bass_kernel_spmd(nc, in_maps, core_ids).results
    import os
    from trn_agent_boot.trn_boot import _ntff_profile_via_ctypes

    hook = _ntff_profile_via_ctypes("/opt/axon/libaxon_pjrt.so")
    # warm-up (NEFF compile) outside the profiled region
    run_bass_kernel_spmd(nc, in_maps, core_ids)
    out_dir = os.path.join(PROFILE_DIR, tag)
    os.makedirs(out_dir, exist_ok=True)
    with hook(out_dir, [0]):
        res = run_bass_kernel_spmd(nc, in_maps, core_ids).results
    return res


def _host_router(x_flat, router_w):
    """fp64 router: logits, softmax, top-2 (lax.top_k tie semantics via
    stable sort), renormalized combine weights [N, E] fp64."""
    logits = x_flat.astype(np.float64) @ router_w.astype(np.float64).T
    logits -= logits.max(axis=1, keepdims=True)
    probs = np.exp(logits)
    probs /= probs.sum(axis=1, keepdims=True)
    order = np.argsort(-probs, axis=1, kind="stable")
    top2 = order[:, :2]                                    # [N, 2]
    rows = np.arange(N)[:, None]
    p2 = probs[rows, top2]                                 # [N, 2]
    p2 /= p2.sum(axis=1, keepdims=True)
    combine = np.zeros((N, E), np.float64)
    combine[rows, top2] = p2
    return combine


def _chunks(T):
    # matmul free dim caps at 512 (one PSUM bank of fp32 output per MM).
    # Near-equal splits: a [512, 512, 7] split pays a ~26ns/instruction
    # issue floor on the 7-wide chunk's 576 matmuls; [344, 344, 343]
    # does the same rows with no floor penalty.
    n = (T + 511) // 512
    base, rem = divmod(T, n)
    return [base + (1 if i < rem else 0) for i in range(n)]


def _build_ffn2(ws):
    """Per core (quarter q, hidden-half h): all 8 experts' segment-q
    tokens against hidden-half h.

      xg   [128, KC, W] f16   partition-blocked gathered x.T (all experts
                              concatenated at offsets cumsum(ws))
      gw/uw [E, MH, 128, KC, 128] f16  half-hidden gate/up weight blocks
      dw   [E, DC, 128, MH, 128] f16   half-hidden down-proj blocks
      wrep [128, W] f32       combine weights replicated over partitions
      outT [128, DC, W] f16   (combine_w * partial expert_out), blocked
    """
    W = sum(ws)
    nc = bacc.Bacc("TRN2", target_bir_lowering=False, debug=False,
                   num_devices=N_CORES)
    xg = nc.dram_tensor("xg", [128, KC, W], F16, kind="ExternalInput").ap()
    gw = nc.dram_tensor("gw", [E, MH, 128, KC, 128], F16,
                        kind="ExternalInput").ap()
    uw = nc.dram_tensor("uw", [E, MH, 128, KC, 128], F16,
                        kind="ExternalInput").ap()
    dw = nc.dram_tensor("dw", [E, DC, 128, MH, 128], F16,
                        kind="ExternalInput").ap()
    wrep = nc.dram_tensor("wrep", [128, W], F32, kind="ExternalInput").ap()
    outT = nc.dram_tensor("outT", [128, DC, W], F16,
                          kind="ExternalOutput").ap()

    with tile.TileContext(nc) as tc:
        with (
            tc.tile_pool(name="xp", bufs=2) as xp,
            tc.tile_pool(name="gp", bufs=5) as gp,
            tc.tile_pool(name="up", bufs=5) as up,
            tc.tile_pool(name="dp", bufs=6) as dp,
            tc.tile_pool(name="hp", bufs=2) as hp,
            tc.tile_pool(name="sg", bufs=2) as sgp,
            tc.tile_pool(name="ot", bufs=4) as otp,
            tc.tile_pool(name="wpl", bufs=2) as wpl,
            tc.tile_pool(name="ps", bufs=2, space="PSUM") as ps,
        ):
            offs = [0]
            for e in range(E):
                offs.append(offs[-1] + ws[e])

            def emit_x(e):
                """x + combine-weight DMAs for expert segment e. For the
                very first chunk, per-k slices so the first chain's k=0
                matmul waits on 130KB, not 2MB."""
                w, off = ws[e], offs[e]
                xt = xp.tile([128, KC, w], F16, tag="x", name=f"x{e}")
                c0 = 0
                for ci, cl in enumerate(_chunks(w)):
                    if e == 0 and ci == 0:
                        for k in range(KC):
                            nc.sync.dma_start(xt[:, k, c0:c0 + cl],
                                              xg[:, k, off:off + cl])
                    else:
                        nc.sync.dma_start(xt[:, :, c0:c0 + cl],
                                          xg[:, :, off + c0:off + c0 + cl])
                    c0 += cl
                wt = wpl.tile([128, w], F32, tag="wt", name=f"wt{e}")
                nc.sync.dma_start(wt[:], wrep[:, off:off + w])
                return xt, wt

            def emit_w01(e, ms=(0, 1)):
                """Lead m-blocks' gate/up weights for segment e —
                emitted ahead so they beat the bulk weight stream.
                DMA'd in k-quarter slices: a chain can start once the
                first 4 k-blocks have landed instead of the whole 1MB."""
                ts = []
                for m in ms:
                    g = gp.tile([128, KC, 128], F16, tag="gw",
                                name=f"gw{e}_{m}")
                    u = up.tile([128, KC, 128], F16, tag="uw",
                                name=f"uw{e}_{m}")
                    for k0 in range(0, KC, 4):
                        nc.sync.dma_start(g[:, k0:k0 + 4, :],
                                          gw[e, m, :, k0:k0 + 4, :])
                        nc.sync.dma_start(u[:, k0:k0 + 4, :],
                                          uw[e, m, :, k0:k0 + 4, :])
                    ts.append((g, u))
                return ts

            # start-up order is bandwidth-critical: interleave segment 0's
            # lead weight m-blocks between its x chunks
            w0 = ws[0]
            ch0 = _chunks(w0)
            pre = emit_w01(0, ms=(0,))
            xt0 = xp.tile([128, KC, w0], F16, tag="x", name="x0")
            for k in range(KC):
                nc.sync.dma_start(xt0[:, k, 0:ch0[0]], xg[:, k, 0:ch0[0]])
            c0 = ch0[0]
            for ci, cl in enumerate(ch0[1:], 1):
                pre += emit_w01(0, ms=(ci,))
                nc.sync.dma_start(xt0[:, :, c0:c0 + cl],
                                  xg[:, :, c0:c0 + cl])
                c0 += cl
            pre += emit_w01(0, ms=(len(ch0),))
            wt0 = wpl.tile([128, w0], F32, tag="wt", name="wt0")
            nc.sync.dma_start(wt0[:], wrep[:, 0:w0])
            xnext = (xt0, wt0)
            for e in range(E):
                w, off = ws[e], offs[e]
                chunks = _chunks(w)
                xt, wt = xnext
                w01 = pre
                ht = hp.tile([128, MH, w], F16, tag="h", name=f"h{e}")
                for m in range(MH):
                    if m < len(w01):
                        gw_t, uw_t = w01[m]
                    else:
                        gw_t = gp.tile([128, KC, 128], F16, tag="gw",
                                       name=f"gw{e}_{m}")
                        nc.sync.dma_start(gw_t[:], gw[e, m])
                        uw_t = up.tile([128, KC, 128], F16, tag="uw",
                                       name=f"uw{e}_{m}")
                        nc.sync.dma_start(uw_t[:], uw[e, m])
                    c0 = 0
                    for ci, cl in enumerate(chunks):
                        gps = ps.tile([128, cl], F32, tag="gps", bufs=2,
                                      name=f"gps{e}_{m}_{ci}")
                        ups = ps.tile([128, cl], F32, tag="ups", bufs=2,
                                      name=f"ups{e}_{m}_{ci}")
                        # interleaved chains: adjacent matmuls have no
                        # accumulation dependency -> deeper PE pipelining.
                        # The very first chunk runs the gate chain alone
                        # so it only waits on gw[0,0] + x slice k=0.
                        if e == 0 and m == 0 and ci == 0:
                            for k in range(KC):
                                nc.tensor.matmul(
                                    gps[:], gw_t[:, k, :],
                                    xt[:, k, c0:c0 + cl],
                                    start=(k == 0), stop=(k == KC - 1))
                            for k in range(KC):
                                nc.tensor.matmul(
                                    ups[:], uw_t[:, k, :],
                                    xt[:, k, c0:c0 + cl],
                                    start=(k == 0), stop=(k == KC - 1))
                        else:
                            for k in range(KC):
                                nc.tensor.matmul(
                                    gps[:], gw_t[:, k, :],
                                    xt[:, k, c0:c0 + cl],
                                    start=(k == 0), stop=(k == KC - 1))
                                nc.tensor.matmul(
                                    ups[:], uw_t[:, k, :],
                                    xt[:, k, c0:c0 + cl],
                                    start=(k == 0), stop=(k == KC - 1))
                        sg = sgp.tile([128, cl], F16, tag="sg",
                                      name=f"sg{e}_{m}_{ci}")
                        nc.scalar.activation(sg[:], gps[:], AF.Silu)
                        nc.vector.tensor_mul(ht[:, m, c0:c0 + cl],
                                             sg[:], ups[:])
                        c0 += cl
                if e + 1 < E:
                    # queue next segment's lead weights + x ahead of this
                    # segment's ~6MB down-weight stream
                    pre = emit_w01(e + 1)
                    xnext = emit_x(e + 1)
                for d0 in range(0, DC, 2):
                    dw_a = dp.tile([128, MH, 128], F16, tag="dw",
                                   name=f"dw{e}_{d0}")
                    nc.sync.dma_start(dw_a[:], dw[e, d0])
                    dw_b = dp.tile([128, MH, 128], F16, tag="dw",
                                   name=f"dw{e}_{d0 + 1}")
                    nc.sync.dma_start(dw_b[:], dw[e, d0 + 1])
                    c0 = 0
                    for ci, cl in enumerate(chunks):
                        opa = ps.tile([128, cl], F32, tag="opa", bufs=2,
                                      name=f"opa{e}_{d0}_{ci}")
                        opb = ps.tile([128, cl], F32, tag="opb", bufs=2,
                                      name=f"opb{e}_{d0}_{ci}")
                        for m in range(MH):
                            nc.tensor.matmul(
                                opa[:], dw_a[:, m, :],
                                ht[:, m, c0:c0 + cl],
                                start=(m == 0), stop=(m == MH - 1))
                            nc.tensor.matmul(
                                opb[:], dw_b[:, m, :],
                                ht[:, m, c0:c0 + cl],
                                start=(m == 0), stop=(m == MH - 1))
                        ota = otp.tile([128, cl], F16, tag="ot",
                                       name=f"ota{e}_{d0}_{ci}")
                        nc.vector.tensor_mul(ota[:], opa[:],
                                             wt[:, c0:c0 + cl])
                        nc.sync.dma_start(
                            outT[:, d0, off + c0:off + c0 + cl], ota[:])
                        otb = otp.tile([128, cl], F16, tag="ot",
                                       name=f"otb{e}_{d0}_{ci}")
                        nc.vector.tensor_mul(otb[:], opb[:],
                                             wt[:, c0:c0 + cl])
                        nc.sync.dma_start(
                            outT[:, d0 + 1, off + c0:off + c0 + cl],
                            otb[:])
                        c0 += cl
    nc.compile()
    return nc


def kernel(x, router_w, gate_w, up_w, down_w):
    x = np.asarray(x, np.float32)
    router_w = np.asarray(router_w, np.float32)
    gate_w = np.asarray(gate_w, np.float32)
    up_w = np.asarray(up_w, np.float32)
    down_w = np.asarray(down_w, np.float32)

    x_flat = np.ascontiguousarray(x.reshape(N, D))

    # ---- host router: decides the dispatch ----
    combine = _host_router(x_flat, router_w)

    # ---- host dispatch: per-expert token lists, split into quarters ----
    idx = [np.flatnonzero(combine[:, e] > 0.0) for e in range(E)]
    cnt = [len(i) for i in idx]
    ws = [(c + NQ - 1) // NQ for c in cnt]   # per-quarter segment width
    W = sum(ws)
    offs = np.cumsum([0] + ws)

    x16 = x_flat.astype(np.float16)
    # gathered x + combine weights per quarter (shared by both h-halves)
    xgs, wreps, qidx = [], [], []
    for q in range(NQ):
        xg = np.zeros((128, KC, W), np.float16)
        wv = np.zeros(W, np.float32)
        qlists = []
        for e in range(E):
            lo = q * ws[e]
            hi = min(lo + ws[e], cnt[e])
            ids = idx[e][lo:hi]
            qlists.append(ids)
            n = len(ids)
            if n:
                seg = x16[ids].reshape(n, KC, 128).transpose(2, 1, 0)
                xg[:, :, offs[e]:offs[e] + n] = seg
                wv[offs[e]:offs[e] + n] = combine[ids, e]
        xgs.append(xg)
        wreps.append(np.ascontiguousarray(
            np.broadcast_to(wv, (128, W))))
        qidx.append(qlists)

    # half-hidden weight blocks per h
    g6 = gate_w.astype(np.float16).reshape(E, NH, MH, 128, KC, 128)
    gwh = np.ascontiguousarray(g6.transpose(1, 0, 2, 5, 4, 3))
    u6 = up_w.astype(np.float16).reshape(E, NH, MH, 128, KC, 128)
    uwh = np.ascontiguousarray(u6.transpose(1, 0, 2, 5, 4, 3))
    d6 = down_w.astype(np.float16).reshape(E, DC, 128, NH, MH, 128)
    dwh = np.ascontiguousarray(d6.transpose(3, 0, 1, 5, 4, 2))

    in_maps = []
    for q in range(NQ):
        for h in range(NH):
            in_maps.append({"xg": xgs[q], "gw": gwh[h], "uw": uwh[h],
                            "dw": dwh[h], "wrep": wreps[q]})

    key = tuple(ws)
    if key not in _cache:
        _cache[key] = _build_ffn2(ws)
    nc_f = _cache[key]
    res = _run(nc_f, in_maps, "launch2")

    # ---- host combine: add hidden-half partials, scatter-add ----
    out = np.zeros((N, D), np.float32)
    for q in range(NQ):
        o = (res[q * NH]["outT"].astype(np.float32)
             + res[q * NH + 1]["outT"].astype(np.float32))
        o = o.transpose(1, 0, 2).reshape(D, W)     # [D, W]
        for e in range(E):
            ids = qidx[q][e]
            n = len(ids)
            if n:
                out[ids] += o[:, offs[e]:offs[e] + n].T
    return out.reshape(B, L, D)
